# revision 1
# baseline (speedup 1.0000x reference)
"""Trainium2 Bass kernel for ComplexMoE (E=4 experts, top-2 routing).

Strategy: data-parallel over the 8192 tokens across 8 NeuronCores (1024
tokens/core); every core computes all 4 experts densely on its tokens and
weights the contributions by the top-2 softmax routing weights (the
reference computes the same dense masked form). Activations stay
feature-major ([feature, token]) so every matmul contraction lands on the
SBUF partition axis with zero on-device transposes; the host pre-permutes
weights/inputs into DMA-identity layouts.

Matmul dtypes: expert matmuls run in float32r (TF32-class, 1 cycle/row,
measured rel-err ~1.5e-4); the tiny router matmuls run in full float32
(4 cycles/row) to keep top-2 selection faithful to the fp32 reference.

Per-core device program (SPMD, no collectives):
  phase A, for chunk in 2 x 512 tokens:
    router: logits [E, 512] = rwT.T @ [xT_r; xT_i] in one fp32 M=4
    group; rows move to partition 0 via tiny SBUF->SBUF DMAs, then
    top-2-of-4 + softmax via a min/max tree on single-partition rows.
    Running both chunks' routing up front lets chunk 1's DVE row chain
    overlap chunk 0's expert matmuls.
  phase B, for chunk in 2 x 512 tokens:
    for e in 4 experts:
      broadcast w_e row -> [128, 512] via ones[1,128].T @ row matmul
      up:   gr/gi/vr/vi [128dh, 512] psum, f32r matmuls, 8 dh-tiles
      gate: sigmoid-based silu(sqrt(gr^2+gi^2+eps)) * w_e ; h = gate * v
      down: yr/yi accumulate 32 f32r matmuls per d-tile; add into SBUF acc
    DMA acc -> HBM
"""

import numpy as np

import concourse.bacc as bacc
import concourse.bass as bass
import concourse.mybir as mybir
import concourse.tile as tile
from concourse.bass_utils import run_bass_kernel_spmd

B, H, T, D = 2, 8, 512, 512
DH = 1024
E = 4
NCORES = 8
NTOK = B * H * T            # 8192
TOKC = NTOK // NCORES       # 1024 tokens per core
KD = D // 128               # 4 k-tiles over D
KH = DH // 128              # 8 k-tiles over DH
CHW = 512                   # token chunk width (one fp32 psum bank)
NCH = TOKC // CHW           # 2 chunks per core

f32 = mybir.dt.float32
f32r = mybir.dt.float32r
ACT = mybir.ActivationFunctionType
ALU = mybir.AluOpType


def _build_bass():
    nc = bacc.Bacc(None)

    # f32r-typed DRAM params hold plain fp32 bits; numpy side sees float32.
    xr = nc.declare_dram_parameter("xr", [128, KD, TOKC], f32r, isOutput=False)
    xi = nc.declare_dram_parameter("xi", [128, KD, TOKC], f32r, isOutput=False)
    xn = nc.declare_dram_parameter("xn", [128, KD, TOKC], f32r, isOutput=False)
    upw = nc.declare_dram_parameter("upw", [E, KH, 128, KD, 4, 128], f32r,
                                    isOutput=False)
    dnw = nc.declare_dram_parameter("dnw", [E, KD, 128, KH, 3, 128], f32r,
                                    isOutput=False)
    rw = nc.declare_dram_parameter("rw", [128, KD * 2, E], f32, isOutput=False)
    oyr = nc.declare_dram_parameter("oyr", [128, KD, TOKC], f32, isOutput=True)
    oyi = nc.declare_dram_parameter("oyi", [128, KD, TOKC], f32, isOutput=True)

    with tile.TileContext(nc) as tc:
        with (
            tc.tile_pool(name="xp", bufs=2) as xp,
            tc.tile_pool(name="xnp", bufs=1) as xnp,
            tc.tile_pool(name="xfp", bufs=1) as xfp,
            tc.tile_pool(name="accp", bufs=1) as accp,
            tc.tile_pool(name="hp", bufs=1) as hp,
            tc.tile_pool(name="wup", bufs=3) as wup,
            tc.tile_pool(name="wdn", bufs=2) as wdn,
            tc.tile_pool(name="gt", bufs=2) as gt,
            tc.tile_pool(name="smalls", bufs=1) as smalls,
            tc.tile_pool(name="wbp", bufs=2) as wbp,
            tc.tile_pool(name="ps", bufs=2, space="PSUM") as ps,
        ):
            rwt = smalls.tile([128, KD * 2, E], f32, tag="rwt")
            nc.sync.dma_start(out=rwt, in_=rw[:, :, :])
            epsb = smalls.tile([128, 1], f32, tag="epsb")
            nc.vector.memset(epsb, 1e-8)
            ones = smalls.tile([1, 128], f32, tag="ones")
            nc.vector.memset(ones, 1.0)

            xtr_l, xti_l, wall = [], [], None
            wall = smalls.tile([1, E, NCH, CHW], f32, tag="wall")
            for ch in range(NCH):
                tsl = slice(ch * CHW, (ch + 1) * CHW)
                # -------- load x chunk (feature-major) --------
                # f32r copies for the expert matmuls (DMA rounds to tf32)...
                xtr = xp.tile([128, KD, CHW], f32r, tag="xtr")
                xti = xp.tile([128, KD, CHW], f32r, tag="xti")
                nc.sync.dma_start(out=xtr, in_=xr[:, :, tsl])
                nc.sync.dma_start(out=xti, in_=xi[:, :, tsl])
                xtr_l.append(xtr)
                xti_l.append(xti)
                # ...and full-fp32 copies for the router: top-2 selection
                # must see unrounded logits or tokens flip experts.
                xr32 = xfp.tile([128, KD, CHW], f32, tag="xr32")
                xi32 = xfp.tile([128, KD, CHW], f32, tag="xi32")
                nc.sync.dma_start(out=xr32, in_=xr[:, :, tsl].bitcast(f32))
                nc.sync.dma_start(out=xi32, in_=xi[:, :, tsl].bitcast(f32))

                # ------- router: logits [E, CHW] in one fp32 M=4 group -----
                rs = smalls.tile([128, 8, CHW], f32, tag="rscr")

                def row(i):
                    return rs[0:1, i, :]

                lg = ps.tile([E, CHW], f32, tag="pa")
                for a in range(KD * 2):
                    xa = xr32 if a < KD else xi32
                    rhs = xa[:, a % KD, :]
                    nc.tensor.matmul(lg, rwt[:, a, :], rhs,
                                     start=(a == 0), stop=(a == KD * 2 - 1))
                lsb = gt.tile([E, CHW], f32, tag="lsb")
                nc.vector.tensor_copy(out=lsb, in_=lg)
                # rows to partition 0 via tiny SBUF->SBUF DMAs (cross-part)
                for e in range(E):
                    nc.sync.dma_start(out=row(e), in_=lsb[e:e + 1, :])
                L = [row(e) for e in range(E)]
                s4, s5, s6, s7 = (row(i) for i in range(4, 8))
                nc.vector.tensor_tensor(s4, L[0], L[1], op=ALU.max)   # m01
                nc.vector.tensor_tensor(s5, L[0], L[1], op=ALU.min)   # n01
                nc.vector.tensor_tensor(s6, L[2], L[3], op=ALU.max)   # m23
                nc.vector.tensor_tensor(s7, L[2], L[3], op=ALU.min)   # n23
                nc.vector.tensor_tensor(s5, s5, s7, op=ALU.max)  # max(n01,n23)
                nc.vector.tensor_tensor(s7, s4, s6, op=ALU.min)  # min(m01,m23)
                nc.vector.tensor_tensor(s4, s4, s6, op=ALU.max)  # m1
                nc.vector.tensor_tensor(s6, s7, s5, op=ALU.max)  # m2
                m1, m2 = s4, s6

                # masked softmax over top-2: w_e = exp(L_e-m1)*[L_e>=m2]/sum
                for e in range(E):
                    we = wall[0:1, e, ch, :]
                    nc.vector.tensor_tensor(s5, L[e], m1, op=ALU.subtract)
                    nc.scalar.activation(out=s5, in_=s5, func=ACT.Exp)
                    nc.vector.tensor_tensor(s7, L[e], m2, op=ALU.is_ge)
                    nc.vector.tensor_tensor(we, s5, s7, op=ALU.mult)
                nc.vector.tensor_tensor(s5, wall[0:1, 0, ch, :],
                                        wall[0:1, 1, ch, :], op=ALU.add)
                nc.vector.tensor_tensor(s5, s5, wall[0:1, 2, ch, :], op=ALU.add)
                nc.vector.tensor_tensor(s5, s5, wall[0:1, 3, ch, :], op=ALU.add)
                nc.vector.reciprocal(out=s7, in_=s5)
                for e in range(E):
                    we = wall[0:1, e, ch, :]
                    nc.vector.tensor_tensor(we, we, s7, op=ALU.mult)

            # -------- phase B: expert compute per chunk --------
            for ch in range(NCH):
                tsl = slice(ch * CHW, (ch + 1) * CHW)
                xtr, xti = xtr_l[ch], xti_l[ch]
                xtn = xnp.tile([128, KD, CHW], f32r, tag="xtn")
                nc.sync.dma_start(out=xtn, in_=xn[:, :, tsl])
                accr = accp.tile([128, KD, CHW], f32, tag="accr")
                acci = accp.tile([128, KD, CHW], f32, tag="acci")

                # -------- expert loop --------
                for e in range(E):
                    # replicate w_e row across 128 partitions: ones.T @ row
                    wbps = ps.tile([128, CHW], f32, tag="pa")
                    nc.tensor.matmul(wbps, ones, wall[0:1, e, ch, :],
                                     start=True, stop=True)
                    wb = wbp.tile([128, CHW], f32, tag="wb")
                    nc.vector.tensor_copy(out=wb, in_=wbps)
                    hr = hp.tile([128, KH, CHW], f32r, tag="hr")
                    hi = hp.tile([128, KH, CHW], f32r, tag="hi")

                    # ---- up projections + gate, one dh-tile at a time ----
                    for j in range(KH):
                        uw = wup.tile([128, KD, 4, 128], f32r, tag="uw")
                        nc.sync.dma_start(out=uw, in_=upw[e, j])
                        gr = ps.tile([128, CHW], f32, tag="pa")
                        gi = ps.tile([128, CHW], f32, tag="pb")
                        vr = ps.tile([128, CHW], f32, tag="pc")
                        vi = ps.tile([128, CHW], f32, tag="pd")
                        for k in range(KD):
                            ugr = uw[:, k, 0, :]
                            ugi = uw[:, k, 1, :]
                            uvr = uw[:, k, 2, :]
                            uvi = uw[:, k, 3, :]
                            ar = xtr[:, k, :]
                            ai = xti[:, k, :]
                            an = xtn[:, k, :]
                            st, sp = (k == 0), (k == KD - 1)
                            # gr = Ugr.T@A + Ugi.T@(-B); gi = Ugi.T@A + Ugr.T@B
                            nc.tensor.matmul(gr, ugr, ar, start=st, stop=False)
                            nc.tensor.matmul(gi, ugr, ai, start=st, stop=False)
                            nc.tensor.matmul(gr, ugi, an, start=False, stop=sp)
                            nc.tensor.matmul(gi, ugi, ar, start=False, stop=sp)
                            nc.tensor.matmul(vr, uvr, ar, start=st, stop=False)
                            nc.tensor.matmul(vi, uvr, ai, start=st, stop=False)
                            nc.tensor.matmul(vr, uvi, an, start=False, stop=sp)
                            nc.tensor.matmul(vi, uvi, ar, start=False, stop=sp)
                        # gate = silu(sqrt(gr^2+gi^2+eps)) * w_e ; h = gate*v
                        t1 = gt.tile([128, CHW], f32, tag="t1")
                        t2 = gt.tile([128, CHW], f32, tag="t2")
                        t3 = gt.tile([128, CHW], f32, tag="t3")
                        nc.scalar.activation(out=t1, in_=gr, func=ACT.Square)
                        nc.scalar.activation(out=t2, in_=gi, func=ACT.Square)
                        nc.vector.tensor_tensor(t3, t1, t2, op=ALU.add)
                        nc.scalar.activation(out=t1, in_=t3, func=ACT.Sqrt,
                                             bias=epsb, scale=1.0)
                        # silu(m) * w_e == (m * w_e) * sigmoid(m)
                        nc.scalar.activation(out=t2, in_=t1, func=ACT.Sigmoid)
                        nc.vector.tensor_tensor(t3, t1, wb, op=ALU.mult)
                        nc.vector.tensor_tensor(t3, t3, t2, op=ALU.mult)
                        nc.vector.tensor_tensor(hr[:, j, :], t3, vr,
                                                op=ALU.mult)
                        nc.vector.tensor_tensor(hi[:, j, :], t3, vi,
                                                op=ALU.mult)

                    # ---- down projection ----
                    for d in range(KD):
                        dw = wdn.tile([128, KH, 3, 128], f32r, tag="dw")
                        nc.sync.dma_start(out=dw, in_=dnw[e, d])
                        yr = ps.tile([128, CHW], f32, tag="pa")
                        yi = ps.tile([128, CHW], f32, tag="pb")
                        for kh in range(KH):
                            dr = dw[:, kh, 0, :]
                            di = dw[:, kh, 1, :]
                            dn_ = dw[:, kh, 2, :]
                            hrk = hr[:, kh, :]
                            hik = hi[:, kh, :]
                            st, sp = (kh == 0), (kh == KH - 1)
                            # yr = Dr.T@Hr + (-Di).T@Hi; yi = Di.T@Hr + Dr.T@Hi
                            nc.tensor.matmul(yr, dr, hrk, start=st, stop=False)
                            nc.tensor.matmul(yi, dr, hik, start=st, stop=False)
                            nc.tensor.matmul(yr, dn_, hik, start=False, stop=sp)
                            nc.tensor.matmul(yi, di, hrk, start=False, stop=sp)
                        if e == 0:
                            nc.vector.tensor_copy(out=accr[:, d, :], in_=yr)
                            nc.vector.tensor_copy(out=acci[:, d, :], in_=yi)
                        else:
                            nc.vector.tensor_tensor(accr[:, d, :],
                                                    accr[:, d, :], yr,
                                                    op=ALU.add)
                            nc.vector.tensor_tensor(acci[:, d, :],
                                                    acci[:, d, :], yi,
                                                    op=ALU.add)

                nc.sync.dma_start(out=oyr[:, :, tsl], in_=accr)
                nc.sync.dma_start(out=oyi[:, :, tsl], in_=acci)
    nc.finalize()
    return nc


_cached_nc = None


def _get_nc():
    global _cached_nc
    if _cached_nc is None:
        _cached_nc = _build_bass()
    return _cached_nc


def _prep_inputs(x_r, x_i, router_w, ug_wr, ug_wi, uv_wr, uv_wi, dn_wr, dn_wi):
    """Host-side layout prep -> per-core input maps."""
    xr2 = np.ascontiguousarray(x_r.reshape(NTOK, D).astype(np.float32))
    xi2 = np.ascontiguousarray(x_i.reshape(NTOK, D).astype(np.float32))

    def upt(w):  # [E, DH, D] -> [E, KH, 128p(D), KD, 128m(DH)]
        return w.reshape(E, KH, 128, KD, 128).transpose(0, 1, 4, 3, 2)

    up = np.ascontiguousarray(
        np.stack([upt(ug_wr), upt(ug_wi), upt(uv_wr), upt(uv_wi)], axis=4),
        dtype=np.float32)  # [E, KH, 128, KD, 4, 128]

    def dnt(w):  # [E, D, DH] -> [E, KD, 128p(DH), KH, 128m(D)]
        return w.reshape(E, KD, 128, KH, 128).transpose(0, 1, 4, 3, 2)

    dr_t, di_t = dnt(dn_wr), dnt(dn_wi)
    dn = np.ascontiguousarray(
        np.stack([dr_t, di_t, -di_t], axis=4), dtype=np.float32)
    rw = np.ascontiguousarray(
        router_w.reshape(E, KD * 2, 128).transpose(2, 1, 0), dtype=np.float32)

    in_maps = []
    for c in range(NCORES):
        sl = slice(c * TOKC, (c + 1) * TOKC)

        def xt(a):  # [TOKC, D] -> [128, KD, TOKC]
            return np.ascontiguousarray(
                a.T.reshape(KD, 128, TOKC).transpose(1, 0, 2))

        xrc = xt(xr2[sl])
        xic = xt(xi2[sl])
        in_maps.append({"xr": xrc, "xi": xic, "xn": np.ascontiguousarray(-xic),
                        "upw": up, "dnw": dn, "rw": rw})
    return in_maps


def run(inputs: dict, trace: bool = False):
    """Returns ((out_r, out_i), BassKernelResults)."""
    assert int(inputs["top_k"]) == 2, "kernel specialized for top_k=2"
    for bname in ("router_b", "ug_br", "ug_bi", "uv_br", "uv_bi", "dn_br",
                  "dn_bi"):
        assert not np.any(np.asarray(inputs[bname])), \
            f"kernel assumes zero bias ({bname})"

    in_maps = _prep_inputs(
        np.asarray(inputs["x_r"], np.float32),
        np.asarray(inputs["x_i"], np.float32),
        np.asarray(inputs["router_w"], np.float32),
        np.asarray(inputs["ug_wr"], np.float32),
        np.asarray(inputs["ug_wi"], np.float32),
        np.asarray(inputs["uv_wr"], np.float32),
        np.asarray(inputs["uv_wi"], np.float32),
        np.asarray(inputs["dn_wr"], np.float32),
        np.asarray(inputs["dn_wi"], np.float32),
    )
    nc = _get_nc()
    res = run_bass_kernel_spmd(nc, in_maps, core_ids=list(range(NCORES)),
                               trace=trace)
    out_r = np.empty((NTOK, D), np.float32)
    out_i = np.empty((NTOK, D), np.float32)
    for c in range(NCORES):
        sl = slice(c * TOKC, (c + 1) * TOKC)
        # [128, KD, TOKC] -> [TOKC, D]
        out_r[sl] = res.results[c]["oyr"].transpose(2, 1, 0).reshape(TOKC, D)
        out_i[sl] = res.results[c]["oyi"].transpose(2, 1, 0).reshape(TOKC, D)
    return (out_r.reshape(B, H, T, D), out_i.reshape(B, H, T, D)), res


def kernel(**inputs):
    (out_r, out_i), _ = run(inputs, trace=False)
    return out_r, out_i



# revision 11
# speedup vs baseline: 1.5200x; 1.5200x over previous
"""Trainium2 Bass kernel for ComplexMoE (E=4 experts, top-2 routing).

v2: expert-parallel dispatch. The host computes top-2 routing (integer
scheduling only), sorts tokens by expert, and ships each expert's tokens
to a pair of cores (core 2e, 2e+1 own expert e). Each core holds its
expert's weights resident in SBUF and runs the full forward for its slot
batch — including the router math (bf16 hi+lo matmul pair for fp32-grade
logits, router weights permuted so row 0 is the core's own expert), the
top-2 softmax weight w = sigmoid(2*l0 - m1 - m2) * [l0 >= m2], the
complex up-projections, silu(|g|) gate, and a 3-matmul Gauss complex
down-projection. Slot outputs are pre-scaled by w; the host adds the two
expert contributions per token. Compute per core is ~2240 slots x 1
expert instead of 1024 tokens x 4 experts -> ~2x fewer FLOPs than dense.

All expert matmuls run in bf16 (fp32 PSUM accumulation; measured rel-err
~4e-3 in numpy simulation vs the fp32 reference, gate is 2e-2). The
router matmul splits fp32 x into bf16 hi+lo parts (x = xh + xl accurate
to 2^-17) so top-2 selection matches the fp32 reference.

Activation-table discipline: Sqrt and Sigmoid live in different table
sets (~2.7us per switch), so the gate is batched per chunk: per-j Square
(in every set, free) -> one big Sqrt -> one big Sigmoid + the router row
sigmoid. GPSIMD (no PSUM port) handles SBUF-only elementwise; vector and
scalar split the PSUM reads.
"""

import numpy as np
import ml_dtypes

import concourse.bacc as bacc
import concourse.bass as bass
import concourse.mybir as mybir
import concourse.tile as tile
from concourse.bass_utils import run_bass_kernel_spmd

B, H, T, D = 2, 8, 512, 512
DH = 1024
E = 4
NCORES = 8
NTOK = B * H * T            # 8192
KD = D // 128               # 4 k-tiles over D
KH = DH // 128              # 8 k-tiles over DH
CHW = 448                   # slot chunk width (<= one fp32 psum bank)
NCH = 5                     # chunks per core
S = CHW * NCH               # 2240 slots per core (max observed need 2058)

f32 = mybir.dt.float32
bf16 = mybir.dt.bfloat16
ACT = mybir.ActivationFunctionType
ALU = mybir.AluOpType
nbf16 = ml_dtypes.bfloat16


def _build_bass():
    nc = bacc.Bacc(None)

    xr = nc.declare_dram_parameter("xr", [128, KD, S], bf16, isOutput=False)
    xi = nc.declare_dram_parameter("xi", [128, KD, S], bf16, isOutput=False)
    xn = nc.declare_dram_parameter("xn", [128, KD, S], bf16, isOutput=False)
    xlr = nc.declare_dram_parameter("xlr", [128, KD, S], bf16, isOutput=False)
    xli = nc.declare_dram_parameter("xli", [128, KD, S], bf16, isOutput=False)
    upw = nc.declare_dram_parameter("upw", [128, KH, KD, 4, 128], bf16,
                                    isOutput=False)
    dnw = nc.declare_dram_parameter("dnw", [128, KD, KH, 3, 128], bf16,
                                    isOutput=False)
    rwh = nc.declare_dram_parameter("rwh", [128, 2 * KD, E], bf16,
                                    isOutput=False)
    rwl = nc.declare_dram_parameter("rwl", [128, 2 * KD, E], bf16,
                                    isOutput=False)
    oyr = nc.declare_dram_parameter("oyr", [128, KD, S], bf16, isOutput=True)
    oyi = nc.declare_dram_parameter("oyi", [128, KD, S], bf16, isOutput=True)

    with tile.TileContext(nc) as tc:
        with (
            tc.tile_pool(name="wres", bufs=1) as wres,
            tc.tile_pool(name="xp", bufs=2) as xp,
            tc.tile_pool(name="xlp", bufs=1) as xlp,
            tc.tile_pool(name="hp", bufs=2) as hp,
            tc.tile_pool(name="vp", bufs=1) as vp,
            tc.tile_pool(name="mp", bufs=1) as mp,
            tc.tile_pool(name="outp", bufs=2) as outp,
            tc.tile_pool(name="gt", bufs=2) as gt,
            tc.tile_pool(name="wbp", bufs=2) as wbp,
            tc.tile_pool(name="smalls", bufs=1) as smalls,
            tc.tile_pool(name="ps", bufs=2, space="PSUM") as ps,
        ):
            # ---- resident weights + constants ----
            upw_s = wres.tile([128, KH, KD, 4, 128], bf16, tag="upw")
            nc.sync.dma_start(out=upw_s, in_=upw[:, :, :, :, :])
            dnw_s = wres.tile([128, KD, KH, 3, 128], bf16, tag="dnw")
            nc.sync.dma_start(out=dnw_s, in_=dnw[:, :, :, :, :])
            rwt_h = smalls.tile([128, 2 * KD, E], bf16, tag="rwh")
            nc.sync.dma_start(out=rwt_h, in_=rwh[:, :, :])
            rwt_l = smalls.tile([128, 2 * KD, E], bf16, tag="rwl")
            nc.sync.dma_start(out=rwt_l, in_=rwl[:, :, :])
            epsb = smalls.tile([128, 1], f32, tag="epsb")
            nc.vector.memset(epsb, 1e-8)
            ones = smalls.tile([1, 128], f32, tag="ones")
            nc.vector.memset(ones, 1.0)
            # router row scratch on partition 0 (8 rows, reused)
            rs = smalls.tile([1, 8, CHW], f32, tag="rs")

            prev = None  # deferred down-phase state from previous chunk

            def emit_down(st):
                xch, hr, hi, hs = st
                tsl = slice(xch * CHW, (xch + 1) * CHW)
                o_r = outp.tile([128, KD, CHW], bf16, tag="or")
                o_i = outp.tile([128, KD, CHW], bf16, tag="oi")
                for dd in range(KD):
                    k1 = ps.tile([128, CHW], f32, tag="pa")
                    k2 = ps.tile([128, CHW], f32, tag="pb")
                    k3 = ps.tile([128, CHW], f32, tag="pc")
                    for kh in range(KH):
                        st_, sp = (kh == 0), (kh == KH - 1)
                        nc.tensor.matmul(k1, dnw_s[:, dd, kh, 0, :],
                                         hs[:, kh, :], start=st_, stop=sp)
                        nc.tensor.matmul(k2, dnw_s[:, dd, kh, 1, :],
                                         hr[:, kh, :], start=st_, stop=sp)
                        nc.tensor.matmul(k3, dnw_s[:, dd, kh, 2, :],
                                         hi[:, kh, :], start=st_, stop=sp)
                    # TT reads at most one PSUM operand: stage k1 in SBUF
                    kc = gt.tile([128, CHW], f32, tag="kc")
                    nc.scalar.copy(out=kc, in_=k1)
                    nc.vector.tensor_tensor(o_r[:, dd, :], kc, k3,
                                            op=ALU.subtract)
                    nc.vector.tensor_tensor(o_i[:, dd, :], kc, k2,
                                            op=ALU.add)
                nc.sync.dma_start(out=oyr[:, :, tsl], in_=o_r)
                nc.sync.dma_start(out=oyi[:, :, tsl], in_=o_i)

            for ch in range(NCH):
                tsl = slice(ch * CHW, (ch + 1) * CHW)
                # -------- load x chunk (feature-major, bf16) --------
                xtr = xp.tile([128, KD, CHW], bf16, tag="xtr")
                xti = xp.tile([128, KD, CHW], bf16, tag="xti")
                xtn = xp.tile([128, KD, CHW], bf16, tag="xtn")
                nc.sync.dma_start(out=xtr, in_=xr[:, :, tsl])
                nc.sync.dma_start(out=xti, in_=xi[:, :, tsl])
                nc.sync.dma_start(out=xtn, in_=xn[:, :, tsl])
                xltr = xlp.tile([128, KD, CHW], bf16, tag="xlr")
                xlti = xlp.tile([128, KD, CHW], bf16, tag="xli")
                nc.sync.dma_start(out=xltr, in_=xlr[:, :, tsl])
                nc.sync.dma_start(out=xlti, in_=xli[:, :, tsl])

                # -------- router: logits [E, CHW] = sum over 8 a-tiles of
                # rwh.T @ xh + rwh.T @ xl + rwl.T @ xh (hi/lo fp32 split) ----
                lg = ps.tile([E, CHW], f32, tag="pa")
                for a in range(2 * KD):
                    xh = (xtr if a < KD else xti)[:, a % KD, :]
                    xl = (xltr if a < KD else xlti)[:, a % KD, :]
                    st_ = (a == 0)
                    sp = (a == 2 * KD - 1)
                    nc.tensor.matmul(lg, rwt_h[:, a, :], xh,
                                     start=st_, stop=False)
                    nc.tensor.matmul(lg, rwt_h[:, a, :], xl,
                                     start=False, stop=False)
                    nc.tensor.matmul(lg, rwt_l[:, a, :], xh,
                                     start=False, stop=sp)
                lsb = gt.tile([E, CHW], f32, tag="lsb")
                nc.vector.tensor_copy(out=lsb, in_=lg)

                def row(i):
                    return rs[0:1, i, :]

                for e in range(E):
                    nc.sync.dma_start(out=row(e), in_=lsb[e:e + 1, :])
                L = [row(e) for e in range(E)]
                s4, s5, s6, s7 = (row(i) for i in range(4, 8))
                nc.vector.tensor_tensor(s4, L[0], L[1], op=ALU.max)
                nc.vector.tensor_tensor(s5, L[0], L[1], op=ALU.min)
                nc.vector.tensor_tensor(s6, L[2], L[3], op=ALU.max)
                nc.vector.tensor_tensor(s7, L[2], L[3], op=ALU.min)
                nc.vector.tensor_tensor(s5, s5, s7, op=ALU.max)  # max(mins)
                nc.vector.tensor_tensor(s7, s4, s6, op=ALU.min)  # min(maxes)
                nc.vector.tensor_tensor(s4, s4, s6, op=ALU.max)  # m1
                nc.vector.tensor_tensor(s6, s7, s5, op=ALU.max)  # m2
                m1r, m2r = s4, s6
                # w0 = [l0 >= m2] * sigmoid(2*l0 - m1 - m2)
                # rows: s5 <- l0-m1, s7 <- l0-m2, tt=L[1] <- sum,
                # ge=L[2] <- mask; sgr=L[3] <- sigmoid (scalar, later)
                ta, tb, tt, ge = s5, s7, L[1], L[2]
                sgr = L[3]
                nc.vector.tensor_tensor(ta, L[0], m1r, op=ALU.subtract)
                nc.vector.tensor_tensor(tb, L[0], m2r, op=ALU.subtract)
                nc.vector.tensor_tensor(tt, ta, tb, op=ALU.add)
                nc.vector.tensor_tensor(ge, L[0], m2r, op=ALU.is_ge)

                # -------- up projections + phase-1 gate pieces --------
                hr = hp.tile([128, KH, CHW], bf16, tag="hr")
                hi = hp.tile([128, KH, CHW], bf16, tag="hi")
                hs = hp.tile([128, KH, CHW], bf16, tag="hs")
                vrs = vp.tile([128, KH, CHW], bf16, tag="vrs")
                vis = vp.tile([128, KH, CHW], bf16, tag="vis")
                m2t = mp.tile([128, KH, CHW], bf16, tag="m2t")
                sgt = mp.tile([128, KH, CHW], bf16, tag="sgt")

                for j in range(KH):
                    gr = ps.tile([128, CHW], f32, tag="pa")
                    gi = ps.tile([128, CHW], f32, tag="pb")
                    vr = ps.tile([128, CHW], f32, tag="pc")
                    vi = ps.tile([128, CHW], f32, tag="pd")
                    for k in range(KD):
                        ugr = upw_s[:, j, k, 0, :]
                        ugi = upw_s[:, j, k, 1, :]
                        uvr = upw_s[:, j, k, 2, :]
                        uvi = upw_s[:, j, k, 3, :]
                        ar = xtr[:, k, :]
                        ai = xti[:, k, :]
                        an = xtn[:, k, :]
                        st_, sp = (k == 0), (k == KD - 1)
                        nc.tensor.matmul(gr, ugr, ar, start=st_, stop=False)
                        nc.tensor.matmul(gi, ugr, ai, start=st_, stop=False)
                        nc.tensor.matmul(gr, ugi, an, start=False, stop=sp)
                        nc.tensor.matmul(gi, ugi, ar, start=False, stop=sp)
                        nc.tensor.matmul(vr, uvr, ar, start=st_, stop=False)
                        nc.tensor.matmul(vi, uvr, ai, start=st_, stop=False)
                        nc.tensor.matmul(vr, uvi, an, start=False, stop=sp)
                        nc.tensor.matmul(vi, uvi, ar, start=False, stop=sp)
                    # phase 1: squares on scalar (Square is in every table
                    # set), PSUM->SBUF v parking split scalar/vector,
                    # m2 add on gpsimd (SBUF only)
                    t1 = gt.tile([128, CHW], f32, tag="t1")
                    t2 = gt.tile([128, CHW], f32, tag="t2")
                    nc.scalar.activation(out=t1, in_=gr, func=ACT.Square)
                    nc.scalar.activation(out=t2, in_=gi, func=ACT.Square)
                    nc.gpsimd.tensor_tensor(m2t[:, j, :], t1, t2, op=ALU.add)
                    nc.scalar.copy(out=vrs[:, j, :], in_=vr)
                    nc.vector.tensor_copy(out=vis[:, j, :], in_=vi)

                # -------- batched gate activations (2 table loads) --------
                nc.scalar.activation(out=m2t[:, :, :], in_=m2t[:, :, :],
                                     func=ACT.Sqrt, bias=epsb, scale=1.0)
                nc.scalar.activation(out=sgt[:, :, :], in_=m2t[:, :, :],
                                     func=ACT.Sigmoid)
                nc.scalar.activation(out=sgr, in_=tt, func=ACT.Sigmoid)

                # -------- deferred down phase of previous chunk --------
                if prev is not None:
                    emit_down(prev)

                # -------- w row + broadcast [128, CHW] --------
                wrow = wbp.tile([1, CHW], f32, tag="wrow")
                nc.vector.tensor_tensor(wrow, sgr, ge, op=ALU.mult)
                wbps = ps.tile([128, CHW], f32, tag="pd")
                nc.tensor.matmul(wbps, ones, wrow,
                                 start=True, stop=True)
                wb = wbp.tile([128, CHW], f32, tag="wb")
                nc.vector.tensor_copy(out=wb, in_=wbps)

                # -------- phase 2: gate * v -> h --------
                for j in range(KH):
                    t3 = gt.tile([128, CHW], f32, tag="t3")
                    nc.vector.tensor_tensor(t3, m2t[:, j, :], wb, op=ALU.mult)
                    nc.vector.tensor_tensor(t3, t3, sgt[:, j, :], op=ALU.mult)
                    nc.gpsimd.tensor_tensor(hr[:, j, :], t3, vrs[:, j, :],
                                            op=ALU.mult)
                    nc.gpsimd.tensor_tensor(hi[:, j, :], t3, vis[:, j, :],
                                            op=ALU.mult)
                    nc.gpsimd.tensor_tensor(hs[:, j, :], hr[:, j, :],
                                            hi[:, j, :], op=ALU.add)

                prev = (ch, hr, hi, hs)

            emit_down(prev)
    nc.finalize()
    return nc


_cached_nc = None


def _get_nc():
    global _cached_nc
    if _cached_nc is None:
        _cached_nc = _build_bass()
    return _cached_nc


def _feat_major(a):
    """[S, D] -> [128, KD, S] (partition = feature mod 128)."""
    return np.ascontiguousarray(a.T.reshape(KD, 128, -1).transpose(1, 0, 2))


def _prep(inputs):
    xr2 = np.ascontiguousarray(
        np.asarray(inputs["x_r"], np.float32).reshape(NTOK, D))
    xi2 = np.ascontiguousarray(
        np.asarray(inputs["x_i"], np.float32).reshape(NTOK, D))
    rw = np.asarray(inputs["router_w"], np.float32)

    # host routing (integer scheduling): top-2 experts per token
    logits = np.concatenate([xr2, xi2], -1) @ rw.T
    order = np.argsort(-logits, axis=-1, kind="stable")
    top2 = order[:, :2]

    # router weights bf16 hi/lo split, [128, 8, E] a-tile layout
    def rlay(x):
        return np.ascontiguousarray(
            np.asarray(x, nbf16).reshape(E, 2 * KD, 128).transpose(2, 1, 0))

    rw_hi = rw.astype(nbf16).astype(np.float32)
    rw_h, rw_l = rlay(rw_hi), rlay(rw - rw_hi)

    in_maps = []
    tok_lists = []
    for c in range(NCORES):
        e = c // 2
        toks = np.where((top2 == e).any(-1))[0]
        n_e = len(toks)
        if n_e > 2 * S:
            raise RuntimeError(f"expert {e} has {n_e} tokens > capacity {2*S}")
        h0 = (n_e + 1) // 2
        part = toks[:h0] if c % 2 == 0 else toks[h0:]
        n_c = len(part)
        tl = np.zeros(S, np.int64)
        tl[:n_c] = part
        tok_lists.append((tl, n_c))

        gxr, gxi = xr2[tl], xi2[tl]
        xh_r = gxr.astype(nbf16)
        xh_i = gxi.astype(nbf16)
        xl_r = (gxr - xh_r.astype(np.float32)).astype(nbf16)
        xl_i = (gxi - xh_i.astype(np.float32)).astype(nbf16)

        # expert weights: up [128, KH, KD, 4, 128], down-Gauss [128, KD, KH, 3, 128]
        def upt(w):  # [DH, D] -> [KH, 128(D part), KD, 128(DH m)]
            return w.reshape(KH, 128, KD, 128).transpose(0, 3, 2, 1)
        up = np.stack([upt(np.asarray(inputs[k][e], np.float32))
                       for k in ("ug_wr", "ug_wi", "uv_wr", "uv_wi")], axis=3)
        up = np.ascontiguousarray(up.transpose(1, 0, 2, 3, 4).astype(nbf16))

        def dnt(w):  # [D, DH] -> [KD, 128(DH part), KH, 128(D m)]
            return w.reshape(KD, 128, KH, 128).transpose(0, 3, 2, 1)
        dwr = np.asarray(inputs["dn_wr"][e], np.float32)
        dwi = np.asarray(inputs["dn_wi"][e], np.float32)
        dn = np.stack([dnt(dwr), dnt(dwi - dwr), dnt(dwi + dwr)], axis=3)
        dn = np.ascontiguousarray(dn.transpose(1, 0, 2, 3, 4).astype(nbf16))

        # router weights permuted: own expert first
        perm = [e] + [x for x in range(E) if x != e]
        in_maps.append({
            "xr": _feat_major(xh_r), "xi": _feat_major(xh_i),
            "xn": _feat_major(np.negative(xh_i)),
            "xlr": _feat_major(xl_r), "xli": _feat_major(xl_i),
            "upw": up, "dnw": dn,
            "rwh": np.ascontiguousarray(rw_h[:, :, perm]),
            "rwl": np.ascontiguousarray(rw_l[:, :, perm]),
        })
    return in_maps, tok_lists


def run(inputs: dict, trace: bool = False):
    assert int(inputs["top_k"]) == 2, "kernel specialized for top_k=2"
    for bname in ("router_b", "ug_br", "ug_bi", "uv_br", "uv_bi", "dn_br",
                  "dn_bi"):
        assert not np.any(np.asarray(inputs[bname])), \
            f"kernel assumes zero bias ({bname})"

    in_maps, tok_lists = _prep(inputs)
    nc = _get_nc()
    res = run_bass_kernel_spmd(nc, in_maps, core_ids=list(range(NCORES)),
                               trace=trace)
    out_r = np.zeros((NTOK, D), np.float32)
    out_i = np.zeros((NTOK, D), np.float32)
    for c in range(NCORES):
        tl, n_c = tok_lists[c]
        yr = res.results[c]["oyr"].transpose(2, 1, 0).reshape(S, D)
        yi = res.results[c]["oyi"].transpose(2, 1, 0).reshape(S, D)
        out_r[tl[:n_c]] += yr[:n_c].astype(np.float32)
        out_i[tl[:n_c]] += yi[:n_c].astype(np.float32)
    return (out_r.reshape(B, H, T, D), out_i.reshape(B, H, T, D)), res


def kernel(**inputs):
    (out_r, out_i), _ = run(inputs, trace=False)
    return out_r, out_i
